# revision 17
# baseline (speedup 1.0000x reference)
"""CompositionAttention Trainium2 kernel.

comp_feat, weight = f(h, roi_feats, context_feat, wh, wv, wa)

Math (per batch b):
  feats[b,n,:] = context[b] - roi[b,n]
  scores[b,n]  = wa . tanh( (h[b] @ wh^T + wh_b + wv_b + context[b] @ wv^T)
                            - roi[b,n] @ wv^T )          (+ wa_b, dropped: softmax-invariant)
  weight[b]    = softmax_n(scores[b])
  comp[b]      = context[b] - sum_n weight[b,n] * roi[b,n]   (since sum_n weight = 1)

Sharding: pure data parallel, batch 256 -> 32 per core across 8 cores.
Weight matrices are packed host-side (transpose + fp16 cast) and replicated.
"""

import sys

if "/opt/trn_rl_repo" not in sys.path:
    sys.path.insert(0, "/opt/trn_rl_repo")

import numpy as np

B, N, RNN, ATT = 256, 196, 1024, 512
NCORES = 8
BC = B // NCORES  # 32 batches per core

_RUNNER = None


def _build_program(bc=BC, n=N, rnn=RNN, att=ATT, gs_cast=True, gs_bcast=True):
    import concourse.bacc as bacc
    import concourse.mybir as mybir
    import concourse.tile as tile

    f32 = mybir.dt.float32
    f16 = mybir.dt.float16
    AFT = mybir.ActivationFunctionType
    ALU = mybir.AluOpType
    X = mybir.AxisListType.X

    rc = rnn // 128  # RNN chunks of 128
    ac = att // 128  # ATT chunks of 128
    n0 = min(128, n)
    n1 = n - n0  # second n block (68 for N=196)
    npairs = bc // 2

    nc = bacc.Bacc("TRN2", target_bir_lowering=False)

    h_d = nc.dram_tensor("h", [bc, rnn], f32, kind="ExternalInput")
    roi_d = nc.dram_tensor("roi_feats", [bc, n, rnn], f32, kind="ExternalInput")
    ctx_d = nc.dram_tensor("context_feat", [bc, rnn], f32, kind="ExternalInput")
    wvT_d = nc.dram_tensor("wvT16", [128, rc, att], f16, kind="ExternalInput")
    whT_d = nc.dram_tensor("whT16", [128, rc, att], f16, kind="ExternalInput")
    waT_d = nc.dram_tensor("waT16", [128, ac], f16, kind="ExternalInput")
    biasT_d = nc.dram_tensor("biasT", [128, ac], f32, kind="ExternalInput")
    eye16_d = nc.dram_tensor("eye16", [128, 128], f16, kind="ExternalInput")
    eye32_d = nc.dram_tensor("eye32", [128, 128], f32, kind="ExternalInput")

    comp_d = nc.dram_tensor("comp", [bc, rnn], f32, kind="ExternalOutput")
    wout_d = nc.dram_tensor("weight", [bc, n], f32, kind="ExternalOutput")

    with tile.TileContext(nc) as tc:
        with tc.tile_pool(name="consts", bufs=1) as consts:
          with (
            tc.tile_pool(name="prep", bufs=1) as prep,
            tc.tile_pool(name="ppre", bufs=2, space="PSUM") as ppre,
          ):
            eye16 = consts.tile([128, 128], f16)
            eye32 = consts.tile([128, 128], f32)
            wvT = consts.tile([128, rc, att], f16)
            whT = consts.tile([128, rc, att], f16)
            waT = consts.tile([128, ac], f16)
            biasT = consts.tile([128, ac], f32)
            ones16 = consts.tile([1, 128], f16)
            nc.vector.memset(ones16[:], 1.0)
            nc.sync.dma_start(eye16[:], eye16_d[:])
            nc.sync.dma_start(eye32[:], eye32_d[:])
            nc.sync.dma_start(wvT[:], wvT_d[:])
            nc.sync.dma_start(whT[:], whT_d[:])
            nc.sync.dma_start(waT[:], waT_d[:])
            nc.sync.dma_start(biasT[:], biasT_d[:])

            # ---- preamble: hT16/cT16 (fp16, [128, rc, bc]) and cT32 ([128, bc, rc]) ----
            h_sb = prep.tile([bc, rnn], f32)
            c_sb = prep.tile([bc, rnn], f32)
            nc.sync.dma_start(h_sb[:], h_d[:])
            nc.sync.dma_start(c_sb[:], ctx_d[:])
            h16 = prep.tile([bc, rnn], f16)
            c16 = prep.tile([bc, rnn], f16)
            nc.vector.tensor_copy(h16[:], h_sb[:])
            nc.vector.tensor_copy(c16[:], c_sb[:])

            hT16 = consts.tile([128, rc, bc], f16)
            cT16 = consts.tile([128, rc, bc], f16)
            cT32 = consts.tile([128, bc, rc], f32)
            for c in range(rc):
                pt_h = ppre.tile([128, bc], f16, tag="pt_h")
                nc.tensor.transpose(pt_h[:], h16[:, 128 * c : 128 * (c + 1)], eye16[:bc, :bc])
                nc.scalar.copy(hT16[:, c, :], pt_h[:])
                pt_c = ppre.tile([128, bc], f16, tag="pt_c")
                nc.tensor.transpose(pt_c[:], c16[:, 128 * c : 128 * (c + 1)], eye16[:bc, :bc])
                nc.scalar.copy(cT16[:, c, :], pt_c[:])
                pt_c32 = ppre.tile([128, bc], f32, tag="pt_c32")
                nc.tensor.transpose(pt_c32[:], c_sb[:, 128 * c : 128 * (c + 1)], eye32[:bc, :bc])
                nc.scalar.copy(cT32[:, :, c], pt_c32[:])

            # ---- pre_T[a_chunk i][p, b] = (wh@h^T + wv@c^T)[128i+p, b] + biasT[p, i] ----
            preT = consts.tile([128, ac, bc], f32)
            for i in range(ac):
                pp = ppre.tile([128, bc], f32, tag="pp")
                for c in range(rc):
                    nc.tensor.matmul(
                        pp[:], whT[:, c, 128 * i : 128 * (i + 1)], hT16[:, c, :],
                        start=(c == 0), stop=False,
                    )
                for c in range(rc):
                    nc.tensor.matmul(
                        pp[:], wvT[:, c, 128 * i : 128 * (i + 1)], cT16[:, c, :],
                        start=False, stop=(c == rc - 1),
                    )
                nc.scalar.activation(
                    preT[:, i, :], pp[:], AFT.Identity,
                    bias=biasT[:, i : i + 1], scale=1.0,
                )

          with (
            tc.tile_pool(name="nat", bufs=4) as natp,
            tc.tile_pool(name="work", bufs=2) as work,
            tc.tile_pool(name="small", bufs=3) as small,
            tc.tile_pool(name="pmm", bufs=1, space="PSUM") as pmm,
            tc.tile_pool(name="ppt", bufs=2 if gs_bcast else 1, space="PSUM") as ppt,
            tc.tile_pool(name="pmisc", bufs=1, space="PSUM") as pmisc,
        ):
            for k in range(npairs):
                # ---- load + cast + transpose roi pair -> roiT [128, rc, 2n] f16 ----
                roiT = work.tile([128, rc, 2 * n], f16, tag="roiT")
                for bl in range(2):
                    b = 2 * k + bl
                    nat0 = natp.tile([n0, rnn], f32, tag="nat0")
                    nc.sync.dma_start(nat0[:], roi_d[b, 0:n0, :])
                    cast_eng = nc.gpsimd if gs_cast else nc.vector
                    nat0_16 = natp.tile([n0, rnn], f16, tag="nat0_16")
                    cast_eng.tensor_copy(nat0_16[:], nat0[:])
                    if n1 > 0:
                        nat1 = natp.tile([n1, rnn], f32, tag="nat1")
                        nc.sync.dma_start(nat1[:], roi_d[b, n0:n, :])
                        nat1_16 = natp.tile([n1, rnn], f16, tag="nat1_16")
                        cast_eng.tensor_copy(nat1_16[:], nat1[:])
                    for c in range(rc):
                        pt = ppt.tile([128, n], f16, tag="pt")
                        nc.tensor.transpose(
                            pt[:, 0:n0], nat0_16[:, 128 * c : 128 * (c + 1)],
                            eye16[:n0, :n0],
                        )
                        if n1 > 0:
                            nc.tensor.transpose(
                                pt[:, n0:n], nat1_16[:, 128 * c : 128 * (c + 1)],
                                eye16[:n1, :n1],
                            )
                        nc.scalar.copy(roiT[:, c, n * bl : n * (bl + 1)], pt[:])

                # ---- stage 1: wvroiT[i] = wv_chunk_i @ roiT  (accumulate over rc) ----
                mm = [pmm.tile([128, 2 * n], f32, tag=f"mm{i}", name=f"mm{i}") for i in range(ac)]
                for i in range(ac):
                    for c in range(rc):
                        nc.tensor.matmul(
                            mm[i][:], wvT[:, c, 128 * i : 128 * (i + 1)], roiT[:, c, :],
                            start=(c == 0), stop=(c == rc - 1),
                        )

                # ---- tanh(pre - wvroi) on ScalarE, per (att-chunk, batch) ----
                th = work.tile([128, ac, 2 * n], f16, tag="th")
                for i in range(ac):
                    for bl in range(2):
                        b = 2 * k + bl
                        nc.scalar.activation(
                            th[:, i, n * bl : n * (bl + 1)],
                            mm[i][:, n * bl : n * (bl + 1)],
                            AFT.Tanh, bias=preT[:, i, b : b + 1], scale=-1.0,
                        )

                # ---- scores[1, (b, n)] = sum_a wa[a] * tanh[a, (b, n)] ----
                ps = pmisc.tile([1, 2 * n], f32, tag="ps")
                for i in range(ac):
                    nc.tensor.matmul(
                        ps[:], waT[:, i : i + 1], th[:, i, :],
                        start=(i == 0), stop=(i == ac - 1),
                    )

                # ---- softmax over n (both batches live along the free dim) ----
                negmax = small.tile([1, 2], f32, tag="negmax")
                nc.vector.tensor_reduce(
                    negmax[:], ps[:].rearrange("p (b n) -> p b n", b=2),
                    X, ALU.max, negate=True,
                )
                shifted = small.tile([1, 2 * n], f32, tag="shifted")
                nc.vector.tensor_tensor(
                    shifted[:].rearrange("p (b n) -> p b n", b=2),
                    ps[:].rearrange("p (b n) -> p b n", b=2),
                    negmax[:].broadcast_to([1, 2, n]),
                    op=ALU.add,
                )
                ex = small.tile([1, 2 * n], f32, tag="ex")
                nc.scalar.activation(ex[:], shifted[:], AFT.Exp)
                ssum = small.tile([1, 2], f32, tag="ssum")
                nc.vector.tensor_reduce(
                    ssum[:], ex[:].rearrange("p (b n) -> p b n", b=2), X, ALU.add
                )
                rinv = small.tile([1, 2], f32, tag="rinv")
                nc.vector.reciprocal(rinv[:], ssum[:])
                wpair = small.tile([1, 2 * n], f32, tag="wpair")
                nc.vector.tensor_tensor(
                    wpair[:].rearrange("p (b n) -> p b n", b=2),
                    ex[:].rearrange("p (b n) -> p b n", b=2),
                    rinv[:].broadcast_to([1, 2, n]),
                    op=ALU.mult,
                )
                nc.sync.dma_start(
                    wout_d[2 * k : 2 * k + 2, :].rearrange("b n -> (b n)"),
                    wpair[:],
                )

                # ---- stage 3: wroiT[p, bl, c] = sum_n w[bl,n] * roiT[p, c, bl*n + n'] ----
                w16 = small.tile([1, 2 * n], f16, tag="w16")
                nc.vector.tensor_copy(w16[:], wpair[:])
                wb16 = small.tile([128, 2, n], f16, tag="wb16")
                if gs_bcast:
                    for bl in range(2):
                        nc.gpsimd.partition_broadcast(
                            wb16[:, bl, :], w16[0:1, n * bl : n * (bl + 1)]
                        )
                else:
                    wbp = pmisc.tile([128, 2 * n], f32, tag="wbp")
                    nc.tensor.matmul(wbp[:], ones16[:], w16[:], start=True, stop=True)
                    nc.scalar.copy(wb16[:].rearrange("p b n -> p (b n)"), wbp[:])

                wroiT = small.tile([128, 2, rc], f32, tag="wroiT")
                for c in range(rc):
                    prod = small.tile([128, 2, n], f32, tag="prod")
                    nc.vector.tensor_tensor(
                        prod[:], roiT[:, c, :].rearrange("p (b n) -> p b n", b=2),
                        wb16[:], op=ALU.mult,
                    )
                    nc.vector.tensor_reduce(wroiT[:, :, c], prod[:], X, ALU.add)

                # ---- comp^T = c^T - wroi^T ; transpose back and store ----
                compT = small.tile([128, 2 * rc], f32, tag="compT")
                nc.vector.tensor_tensor(
                    compT[:].rearrange("p (b c) -> p b c", b=2),
                    cT32[:, 2 * k : 2 * k + 2, :], wroiT[:], op=ALU.subtract,
                )
                pct = pmisc.tile([2 * rc, 128], f32, tag="pct")
                nc.tensor.transpose(pct[:], compT[:], eye32[:])
                comp_sb = small.tile([2 * rc, 128], f32, tag="comp_sb")
                nc.scalar.copy(comp_sb[:], pct[:])
                nc.sync.dma_start(
                    comp_d[2 * k : 2 * k + 2, :].rearrange("b (c d) -> (b c) d", d=128),
                    comp_sb[:],
                )

    nc.compile()
    return nc


def _pack_weights(wh_w, wh_b, wv_w, wv_b, wa_w):
    rc, ac = RNN // 128, ATT // 128
    # wT16[p, c, a] = w[a, 128c + p]
    wvT16 = np.ascontiguousarray(
        wv_w.T.reshape(rc, 128, ATT).transpose(1, 0, 2).astype(np.float16)
    )
    whT16 = np.ascontiguousarray(
        wh_w.T.reshape(rc, 128, ATT).transpose(1, 0, 2).astype(np.float16)
    )
    # waT16[p, i] = wa[0, 128i + p]
    waT16 = np.ascontiguousarray(wa_w[0].reshape(ac, 128).T.astype(np.float16))
    biasT = np.ascontiguousarray(
        (wh_b + wv_b).reshape(ac, 128).T.astype(np.float32)
    )
    eye16 = np.eye(128, dtype=np.float16)
    eye32 = np.eye(128, dtype=np.float32)
    return dict(
        wvT16=wvT16, whT16=whT16, waT16=waT16, biasT=biasT,
        eye16=eye16, eye32=eye32,
    )


def _get_runner():
    """Build the bass program once and return a cached jitted runner."""
    global _RUNNER
    if _RUNNER is not None:
        return _RUNNER

    import jax
    import numpy as _np
    from jax.sharding import Mesh, PartitionSpec
    from jax.experimental.shard_map import shard_map
    import concourse.mybir as mybir
    from concourse import bass2jax

    nc = _build_program()
    bass2jax.install_neuronx_cc_hook()

    partition_name = nc.partition_id_tensor.name if nc.partition_id_tensor else None

    in_names: list[str] = []
    out_names: list[str] = []
    out_avals = []
    zero_shapes = []
    for alloc in nc.m.functions[0].allocations:
        if not isinstance(alloc, mybir.MemoryLocationSet):
            continue
        name = alloc.memorylocations[0].name
        if alloc.kind == "ExternalInput":
            if name != partition_name:
                in_names.append(name)
        elif alloc.kind == "ExternalOutput":
            out_names.append(name)
            shape = tuple(alloc.tensor_shape)
            dtype = mybir.dt.np(alloc.dtype)
            out_avals.append(jax.core.ShapedArray(shape, dtype))
            zero_shapes.append((shape, dtype))

    n_params = len(in_names)
    n_outs = len(out_names)
    all_names = tuple(in_names + out_names)
    if partition_name is not None:
        all_names = all_names + (partition_name,)

    # Everything is sharded on axis 0 (run_bass_via_pjrt-style): batch inputs
    # are already global; replicated weights get tiled 8x on axis 0 in run().
    batch_inputs = {"h", "roi_feats", "context_feat"}

    def _body(*args):
        operands = list(args)
        if partition_name is not None:
            operands.append(bass2jax.partition_id_tensor())
        outs = bass2jax._bass_exec_p.bind(
            *operands,
            out_avals=tuple(out_avals),
            in_names=all_names,
            out_names=tuple(out_names),
            lowering_input_output_aliases=(),
            sim_require_finite=True,
            sim_require_nnan=True,
            nc=nc,
        )
        return tuple(outs)

    devices = jax.devices()[:NCORES]
    mesh = Mesh(_np.asarray(devices), ("core",))
    in_specs = (PartitionSpec("core"),) * (n_params + n_outs)
    out_specs = (PartitionSpec("core"),) * n_outs
    donate = tuple(range(n_params, n_params + n_outs))
    sharded = jax.jit(
        shard_map(_body, mesh=mesh, in_specs=in_specs, out_specs=out_specs,
                  check_rep=False),
        donate_argnums=donate, keep_unused=True,
    )

    def _global_args(feed: dict):
        args = []
        for nm in in_names:
            a = feed[nm]
            if nm not in batch_inputs:
                a = _np.concatenate([a] * NCORES, axis=0)
            args.append(a)
        return args

    def run(feed: dict):
        zeros = [
            _np.zeros((NCORES * s[0], *s[1:]), dt) for (s, dt) in zero_shapes
        ]
        outs = sharded(*_global_args(feed), *zeros)
        return {nm: _np.asarray(o) for nm, o in zip(out_names, outs)}

    def bench(feed: dict, iters: int = 10):
        """Time steady-state executions with device-resident inputs."""
        import time

        dev_args = [jax.device_put(a) for a in _global_args(feed)]
        def one():
            zeros = [
                _np.zeros((NCORES * s[0], *s[1:]), dt) for (s, dt) in zero_shapes
            ]
            outs = sharded(*dev_args, *zeros)
            jax.block_until_ready(outs)
            return outs

        one()  # warm
        times = []
        for _ in range(iters):
            t0 = time.perf_counter()
            one()
            times.append(time.perf_counter() - t0)
        return min(times), times

    run.bench = bench
    _RUNNER = run
    return run


def kernel(h, roi_feats, context_feat, wh_w, wh_b, wv_w, wv_b, wa_w, wa_b):
    h = np.ascontiguousarray(np.asarray(h, dtype=np.float32))
    roi_feats = np.ascontiguousarray(np.asarray(roi_feats, dtype=np.float32))
    context_feat = np.ascontiguousarray(np.asarray(context_feat, dtype=np.float32))

    feed = _pack_weights(
        np.asarray(wh_w, np.float32), np.asarray(wh_b, np.float32),
        np.asarray(wv_w, np.float32), np.asarray(wv_b, np.float32),
        np.asarray(wa_w, np.float32),
    )
    feed["h"] = h
    feed["roi_feats"] = roi_feats
    feed["context_feat"] = context_feat

    run = _get_runner()
    outs = run(feed)
    return outs["comp"], outs["weight"]


# revision 30
# speedup vs baseline: 1.0209x; 1.0209x over previous
"""CompositionAttention Trainium2 kernel.

comp_feat, weight = f(h, roi_feats, context_feat, wh, wv, wa)

Math (per batch b):
  feats[b,n,:] = context[b] - roi[b,n]
  scores[b,n]  = wa . tanh( (h[b] @ wh^T + wh_b + wv_b + context[b] @ wv^T)
                            - roi[b,n] @ wv^T )          (+ wa_b, dropped: softmax-invariant)
  weight[b]    = softmax_n(scores[b])
  comp[b]      = context[b] - sum_n weight[b,n] * roi[b,n]   (since sum_n weight = 1)

Sharding: pure data parallel, batch 256 -> 32 per core across 8 cores.
Weight matrices are packed host-side (transpose + fp16 cast) and replicated.
"""

import sys

if "/opt/trn_rl_repo" not in sys.path:
    sys.path.insert(0, "/opt/trn_rl_repo")

import numpy as np

B, N, RNN, ATT = 256, 196, 1024, 512
NCORES = 8
BC = B // NCORES  # 32 batches per core

_RUNNER = None


def _build_program(bc=BC, n=N, rnn=RNN, att=ATT, gs_cast=True, gs_bcast=True,
                   repeat=1):
    import concourse.bacc as bacc
    import concourse.mybir as mybir
    import concourse.tile as tile

    f32 = mybir.dt.float32
    f16 = mybir.dt.float16
    AFT = mybir.ActivationFunctionType
    ALU = mybir.AluOpType
    X = mybir.AxisListType.X

    rc = rnn // 128  # RNN chunks of 128
    ac = att // 128  # ATT chunks of 128
    n0 = min(128, n)
    n1 = n - n0  # second n block (68 for N=196)
    npairs = bc // 2

    nc = bacc.Bacc("TRN2", target_bir_lowering=False)

    h_d = nc.dram_tensor("h", [bc, rnn], f32, kind="ExternalInput")
    roi_d = nc.dram_tensor("roi_feats", [bc, n, rnn], f32, kind="ExternalInput")
    ctx_d = nc.dram_tensor("context_feat", [bc, rnn], f32, kind="ExternalInput")
    wvT_d = nc.dram_tensor("wvT16", [128, rc, att], f16, kind="ExternalInput")
    whT_d = nc.dram_tensor("whT16", [128, rc, att], f16, kind="ExternalInput")
    waT_d = nc.dram_tensor("waT16", [128, ac], f16, kind="ExternalInput")
    biasT_d = nc.dram_tensor("biasT", [128, ac], f32, kind="ExternalInput")
    eye16_d = nc.dram_tensor("eye16", [128, 128], f16, kind="ExternalInput")
    eye32_d = nc.dram_tensor("eye32", [128, 128], f32, kind="ExternalInput")

    comp_d = nc.dram_tensor("comp", [bc, rnn], f32, kind="ExternalOutput")
    wout_d = nc.dram_tensor("weight", [bc, n], f32, kind="ExternalOutput")

    with tile.TileContext(nc) as tc:
        with tc.tile_pool(name="consts", bufs=1) as consts:
          with (
            tc.tile_pool(name="prep", bufs=1) as prep,
            tc.tile_pool(name="ppre", bufs=2, space="PSUM") as ppre,
          ):
            eye16 = consts.tile([128, 128], f16)
            eye32 = consts.tile([128, 128], f32)
            wvT = consts.tile([128, rc, att], f16)
            whT = consts.tile([128, rc, att], f16)
            waT = consts.tile([128, ac], f16)
            biasT = consts.tile([128, ac], f32)
            ones16 = consts.tile([1, 128], f16)
            nc.vector.memset(ones16[:], 1.0)
            nc.sync.dma_start(eye16[:], eye16_d[:])
            nc.scalar.dma_start(eye32[:], eye32_d[:])
            nc.scalar.dma_start(wvT[:], wvT_d[:])
            nc.scalar.dma_start(whT[:], whT_d[:])
            nc.scalar.dma_start(waT[:], waT_d[:])
            nc.scalar.dma_start(biasT[:], biasT_d[:])

            # ---- preamble: hT16/cT16 (fp16, [128, rc, bc]) and cT32 ([128, bc, rc]) ----
            h_sb = prep.tile([bc, rnn], f32)
            c_sb = consts.tile([bc, rnn], f32)
            nc.scalar.dma_start(h_sb[:], h_d[:])
            nc.scalar.dma_start(c_sb[:], ctx_d[:])
            h16 = prep.tile([bc, rnn], f16)
            c16 = prep.tile([bc, rnn], f16)
            nc.vector.tensor_copy(h16[:], h_sb[:])
            nc.vector.tensor_copy(c16[:], c_sb[:])

            hT16 = consts.tile([128, rc, bc], f16)
            cT16 = consts.tile([128, rc, bc], f16)
            cT32 = consts.tile([128, bc, rc], f32)
            for c in range(rc):
                pt_h = ppre.tile([128, bc], f16, tag="pt_h")
                nc.tensor.transpose(pt_h[:], h16[:, 128 * c : 128 * (c + 1)], eye16[:bc, :bc])
                nc.scalar.copy(hT16[:, c, :], pt_h[:])
                pt_c = ppre.tile([128, bc], f16, tag="pt_c")
                nc.tensor.transpose(pt_c[:], c16[:, 128 * c : 128 * (c + 1)], eye16[:bc, :bc])
                nc.scalar.copy(cT16[:, c, :], pt_c[:])
                pt_c32 = ppre.tile([128, bc], f32, tag="pt_c32")
                nc.tensor.transpose(pt_c32[:], c_sb[:, 128 * c : 128 * (c + 1)], eye32[:bc, :bc])
                nc.scalar.copy(cT32[:, :, c], pt_c32[:])

            # ---- pre_T[a_chunk i][p, b] = (wh@h^T + wv@c^T)[128i+p, b] + biasT[p, i] ----
            preT = consts.tile([128, ac, bc], f32)
            for i in range(ac):
                pp = ppre.tile([128, bc], f32, tag="pp")
                for c in range(rc):
                    nc.tensor.matmul(
                        pp[:], whT[:, c, 128 * i : 128 * (i + 1)], hT16[:, c, :],
                        start=(c == 0), stop=False,
                    )
                for c in range(rc):
                    nc.tensor.matmul(
                        pp[:], wvT[:, c, 128 * i : 128 * (i + 1)], cT16[:, c, :],
                        start=False, stop=(c == rc - 1),
                    )
                nc.scalar.activation(
                    preT[:, i, :], pp[:], AFT.Identity,
                    bias=biasT[:, i : i + 1], scale=1.0,
                )

          with (
            tc.tile_pool(name="nat", bufs=4) as natp,
            tc.tile_pool(name="nat16", bufs=8) as natp16,
            tc.tile_pool(name="work", bufs=2) as work,
            tc.tile_pool(name="small", bufs=5) as small,
            tc.tile_pool(name="pmm", bufs=2, space="PSUM") as pmm,
            tc.tile_pool(name="ppt", bufs=2, space="PSUM") as ppt,
            tc.tile_pool(name="pmisc", bufs=1, space="PSUM") as pmisc,
          ):
            cast_eng = nc.gpsimd if gs_cast else nc.vector
            segs = [(0, n0, 0), (n0, n1, 1), (n, n0, 2), (n + n0, n1, 3)]

            # chunks per psum bank for the transpose staging tile
            GRP = max(1, min(rc, 2048 // (2 * n * 2)))

            def emit_A(k):
                """load + cast + transpose + stage1 + tanh + scores + softmax."""
                roiT = work.tile([128, rc, 2 * n], f16, tag="roiT", name="roiT")
                # one DMA per n-block for the whole pair: [p, bl, r]
                nat0 = natp.tile([n0, 2, rnn], f32, tag="nat0", name="nat0")
                nc.sync.dma_start(
                    nat0[:], roi_d[2 * k : 2 * k + 2, 0:n0, :].rearrange("b p r -> p b r")
                )
                nat0_16 = natp16.tile([n0, 2, rnn], f16, tag="nat0_16", name="nat0_16")
                cast_eng.tensor_copy(nat0_16[:], nat0[:])
                nat1_16 = None
                if n1 > 0:
                    nat1 = natp.tile([n1, 2, rnn], f32, tag="nat1", name="nat1")
                    nc.sync.dma_start(
                        nat1[:], roi_d[2 * k : 2 * k + 2, n0:n, :].rearrange("b p r -> p b r")
                    )
                    nat1_16 = natp16.tile([n1, 2, rnn], f16, tag="nat1_16", name="nat1_16")
                    cast_eng.tensor_copy(nat1_16[:], nat1[:])
                nat16s = (nat0_16, nat1_16)
                for bl in range(2):
                    for g in range(0, rc, GRP):
                        ng = min(GRP, rc - g)
                        pt = ppt.tile([128, GRP, n], f16, tag="pt", name="pt")
                        for cc in range(ng):
                            c = g + cc
                            nc.tensor.transpose(
                                pt[:, cc, 0:n0],
                                nat0_16[:, bl, 128 * c : 128 * (c + 1)],
                                eye16[:n0, :n0],
                            )
                            if n1 > 0:
                                nc.tensor.transpose(
                                    pt[:, cc, n0:n],
                                    nat1_16[:, bl, 128 * c : 128 * (c + 1)],
                                    eye16[:n1, :n1],
                                )
                        dst = roiT[:, g : g + ng, n * bl : n * (bl + 1)]
                        if (bl + g) % 2 == 0:
                            nc.scalar.copy(dst, pt[:, 0:ng, :])
                        else:
                            nc.vector.tensor_copy(dst, pt[:, 0:ng, :])

                # stage 1: wvroiT[i] = wv_chunk_i @ roiT (accumulate over rc)
                th = work.tile([128, ac, 2 * n], f16, tag="th", name="th")
                for i in range(ac):
                    mm = pmm.tile([128, 2 * n], f32, tag="mm", name="mm")
                    for c in range(rc):
                        nc.tensor.matmul(
                            mm[:], wvT[:, c, 128 * i : 128 * (i + 1)], roiT[:, c, :],
                            start=(c == 0), stop=(c == rc - 1),
                        )
                    for bl in range(2):
                        b = 2 * k + bl
                        nc.scalar.activation(
                            th[:, i, n * bl : n * (bl + 1)],
                            mm[:, n * bl : n * (bl + 1)],
                            AFT.Tanh, bias=preT[:, i, b : b + 1], scale=-1.0,
                        )

                # scores[1, (b, n)] = sum_a wa[a] * tanh[a, (b, n)]
                ps = pmisc.tile([1, 2 * n], f32, tag="ps", name="ps")
                for i in range(ac):
                    nc.tensor.matmul(
                        ps[:], waT[:, i : i + 1], th[:, i, :],
                        start=(i == 0), stop=(i == ac - 1),
                    )

                # softmax over n (both batches along the free dim)
                negmax = small.tile([1, 2], f32, tag="negmax", name="negmax")
                nc.vector.tensor_reduce(
                    negmax[:], ps[:].rearrange("p (b n) -> p b n", b=2),
                    X, ALU.max, negate=True,
                )
                shifted = small.tile([1, 2 * n], f32, tag="shifted", name="shifted")
                nc.vector.tensor_tensor(
                    shifted[:].rearrange("p (b n) -> p b n", b=2),
                    ps[:].rearrange("p (b n) -> p b n", b=2),
                    negmax[:].broadcast_to([1, 2, n]),
                    op=ALU.add,
                )
                ex = small.tile([1, 2 * n], f32, tag="ex", name="ex")
                nc.scalar.activation(ex[:], shifted[:], AFT.Exp)
                ssum = small.tile([1, 2], f32, tag="ssum", name="ssum")
                nc.vector.tensor_reduce(
                    ssum[:], ex[:].rearrange("p (b n) -> p b n", b=2), X, ALU.add
                )
                rinv = small.tile([1, 2], f32, tag="rinv", name="rinv")
                nc.vector.reciprocal(rinv[:], ssum[:])
                wpair = small.tile([1, 2 * n], f32, tag="wpair", name="wpair")
                nc.vector.tensor_tensor(
                    wpair[:].rearrange("p (b n) -> p b n", b=2),
                    ex[:].rearrange("p (b n) -> p b n", b=2),
                    rinv[:].broadcast_to([1, 2, n]),
                    op=ALU.mult,
                )
                nc.scalar.dma_start(
                    wout_d[2 * k : 2 * k + 2, :].rearrange("b n -> (b n)"),
                    wpair[:],
                )
                w16 = small.tile([1, 2 * n], f16, tag="w16", name="w16")
                nc.vector.tensor_copy(w16[:], wpair[:])
                return (k, nat16s, w16)

            def emit_B(st):
                """weighted sum of roi on PE (transposed out) + comp output."""
                k, nat16s, w16 = st
                # transpose the 4 weight segments onto partitions (psum cols)
                pw = pmisc.tile([128, 4, 2], f16, tag="pw", name="pw")
                for s0, ln, j in segs:
                    nc.tensor.transpose(
                        pw[0:ln, j, 0:1], w16[0:1, s0 : s0 + ln],
                        eye16[0:1, 0:1],
                    )
                wsb = small.tile([128, 4], f16, tag="wsb", name="wsb")
                for s0, ln, j in segs:
                    nc.vector.tensor_copy(wsb[0:ln, j : j + 1], pw[0:ln, j, 0:1])

                # wroiT[:, bl, c] = nat16[bl][:, c-chunk].T @ w_col[bl]
                wroiT = pmisc.tile([128, 2, rc], f32, tag="wroiT", name="wroiT")
                nat0_16, nat1_16 = nat16s
                for bl in range(2):
                    for c in range(rc):
                        nc.tensor.matmul(
                            wroiT[:, bl, c : c + 1],
                            nat0_16[:, bl, 128 * c : 128 * (c + 1)],
                            wsb[0:n0, 2 * bl : 2 * bl + 1],
                            start=True, stop=(n1 == 0),
                        )
                        if n1 > 0:
                            nc.tensor.matmul(
                                wroiT[:, bl, c : c + 1],
                                nat1_16[:, bl, 128 * c : 128 * (c + 1)],
                                wsb[0:n1, 2 * bl + 1 : 2 * bl + 2],
                                start=False, stop=True,
                            )

                # comp^T = c^T - wroi^T ; transpose back and store
                compT = small.tile([128, 2 * rc], f32, tag="compT", name="compT")
                nc.vector.tensor_tensor(
                    compT[:].rearrange("p (b c) -> p b c", b=2),
                    cT32[:, 2 * k : 2 * k + 2, :], wroiT[:], op=ALU.subtract,
                )
                pct = pmisc.tile([2 * rc, 128], f32, tag="pct", name="pct")
                nc.tensor.transpose(pct[:], compT[:], eye32[:])
                comp_sb = small.tile([2 * rc, 128], f32, tag="comp_sb", name="comp_sb")
                nc.scalar.copy(comp_sb[:], pct[:])
                nc.scalar.dma_start(
                    comp_d[2 * k : 2 * k + 2, :].rearrange("b (c d) -> (b c) d", d=128),
                    comp_sb[:],
                )

            pending = []
            for k in [kk for _ in range(repeat) for kk in range(npairs)]:
                st = emit_A(k)
                pending.append(st)
                if len(pending) > 2:
                    emit_B(pending.pop(0))
            while pending:
                emit_B(pending.pop(0))

    nc.compile()
    return nc


def _pack_weights(wh_w, wh_b, wv_w, wv_b, wa_w):
    rc, ac = RNN // 128, ATT // 128
    # wT16[p, c, a] = w[a, 128c + p]
    wvT16 = np.ascontiguousarray(
        wv_w.T.reshape(rc, 128, ATT).transpose(1, 0, 2).astype(np.float16)
    )
    whT16 = np.ascontiguousarray(
        wh_w.T.reshape(rc, 128, ATT).transpose(1, 0, 2).astype(np.float16)
    )
    # waT16[p, i] = wa[0, 128i + p]
    waT16 = np.ascontiguousarray(wa_w[0].reshape(ac, 128).T.astype(np.float16))
    biasT = np.ascontiguousarray(
        (wh_b + wv_b).reshape(ac, 128).T.astype(np.float32)
    )
    eye16 = np.eye(128, dtype=np.float16)
    eye32 = np.eye(128, dtype=np.float32)
    return dict(
        wvT16=wvT16, whT16=whT16, waT16=waT16, biasT=biasT,
        eye16=eye16, eye32=eye32,
    )


def _get_runner():
    """Build the bass program once and return a cached jitted runner."""
    global _RUNNER
    if _RUNNER is not None:
        return _RUNNER

    import jax
    import numpy as _np
    from jax.sharding import Mesh, PartitionSpec
    from jax.experimental.shard_map import shard_map
    import concourse.mybir as mybir
    from concourse import bass2jax

    nc = _build_program()
    bass2jax.install_neuronx_cc_hook()

    partition_name = nc.partition_id_tensor.name if nc.partition_id_tensor else None

    in_names: list[str] = []
    out_names: list[str] = []
    out_avals = []
    zero_shapes = []
    for alloc in nc.m.functions[0].allocations:
        if not isinstance(alloc, mybir.MemoryLocationSet):
            continue
        name = alloc.memorylocations[0].name
        if alloc.kind == "ExternalInput":
            if name != partition_name:
                in_names.append(name)
        elif alloc.kind == "ExternalOutput":
            out_names.append(name)
            shape = tuple(alloc.tensor_shape)
            dtype = mybir.dt.np(alloc.dtype)
            out_avals.append(jax.core.ShapedArray(shape, dtype))
            zero_shapes.append((shape, dtype))

    n_params = len(in_names)
    n_outs = len(out_names)
    all_names = tuple(in_names + out_names)
    if partition_name is not None:
        all_names = all_names + (partition_name,)

    # Everything is sharded on axis 0 (run_bass_via_pjrt-style): batch inputs
    # are already global; replicated weights get tiled 8x on axis 0 in run().
    batch_inputs = {"h", "roi_feats", "context_feat"}

    def _body(*args):
        operands = list(args)
        if partition_name is not None:
            operands.append(bass2jax.partition_id_tensor())
        outs = bass2jax._bass_exec_p.bind(
            *operands,
            out_avals=tuple(out_avals),
            in_names=all_names,
            out_names=tuple(out_names),
            lowering_input_output_aliases=(),
            sim_require_finite=True,
            sim_require_nnan=True,
            nc=nc,
        )
        return tuple(outs)

    devices = jax.devices()[:NCORES]
    mesh = Mesh(_np.asarray(devices), ("core",))
    in_specs = (PartitionSpec("core"),) * (n_params + n_outs)
    out_specs = (PartitionSpec("core"),) * n_outs
    donate = tuple(range(n_params, n_params + n_outs))
    sharded = jax.jit(
        shard_map(_body, mesh=mesh, in_specs=in_specs, out_specs=out_specs,
                  check_rep=False),
        donate_argnums=donate, keep_unused=True,
    )

    def _global_args(feed: dict):
        args = []
        for nm in in_names:
            a = feed[nm]
            if nm not in batch_inputs:
                a = _np.concatenate([a] * NCORES, axis=0)
            args.append(a)
        return args

    def run(feed: dict):
        zeros = [
            _np.zeros((NCORES * s[0], *s[1:]), dt) for (s, dt) in zero_shapes
        ]
        outs = sharded(*_global_args(feed), *zeros)
        return {nm: _np.asarray(o) for nm, o in zip(out_names, outs)}

    def bench(feed: dict, iters: int = 10):
        """Time steady-state executions with device-resident inputs."""
        import time

        dev_args = [jax.device_put(a) for a in _global_args(feed)]
        def one():
            zeros = [
                _np.zeros((NCORES * s[0], *s[1:]), dt) for (s, dt) in zero_shapes
            ]
            outs = sharded(*dev_args, *zeros)
            jax.block_until_ready(outs)
            return outs

        one()  # warm
        times = []
        for _ in range(iters):
            t0 = time.perf_counter()
            one()
            times.append(time.perf_counter() - t0)
        return min(times), times

    run.bench = bench
    _RUNNER = run
    return run


def kernel(h, roi_feats, context_feat, wh_w, wh_b, wv_w, wv_b, wa_w, wa_b):
    h = np.ascontiguousarray(np.asarray(h, dtype=np.float32))
    roi_feats = np.ascontiguousarray(np.asarray(roi_feats, dtype=np.float32))
    context_feat = np.ascontiguousarray(np.asarray(context_feat, dtype=np.float32))

    feed = _pack_weights(
        np.asarray(wh_w, np.float32), np.asarray(wh_b, np.float32),
        np.asarray(wv_w, np.float32), np.asarray(wv_b, np.float32),
        np.asarray(wa_w, np.float32),
    )
    feed["h"] = h
    feed["roi_feats"] = roi_feats
    feed["context_feat"] = context_feat

    run = _get_runner()
    outs = run(feed)
    return outs["comp"], outs["weight"]


# revision 31
# speedup vs baseline: 331.1247x; 324.3563x over previous
"""CompositionAttention Trainium2 kernel.

comp_feat, weight = f(h, roi_feats, context_feat, wh, wv, wa)

Math (per batch b):
  feats[b,n,:] = context[b] - roi[b,n]
  scores[b,n]  = wa . tanh( (h[b] @ wh^T + wh_b + wv_b + context[b] @ wv^T)
                            - roi[b,n] @ wv^T )          (+ wa_b, dropped: softmax-invariant)
  weight[b]    = softmax_n(scores[b])
  comp[b]      = context[b] - sum_n weight[b,n] * roi[b,n]   (since sum_n weight = 1)

Sharding: pure data parallel, batch 256 -> 32 per core across 8 cores.
Weight matrices are packed host-side (transpose + fp16 cast) and replicated.
"""

import sys

if "/opt/trn_rl_repo" not in sys.path:
    sys.path.insert(0, "/opt/trn_rl_repo")

import numpy as np

B, N, RNN, ATT = 256, 196, 1024, 512
NCORES = 8
BC = B // NCORES  # 32 batches per core

_RUNNER = None


def _build_program(bc=BC, n=N, rnn=RNN, att=ATT, gs_cast=True, gs_bcast=True,
                   repeat=1, hw_loop=0):
    import concourse.bacc as bacc
    import concourse.mybir as mybir
    import concourse.tile as tile

    f32 = mybir.dt.float32
    f16 = mybir.dt.float16
    AFT = mybir.ActivationFunctionType
    ALU = mybir.AluOpType
    X = mybir.AxisListType.X

    rc = rnn // 128  # RNN chunks of 128
    ac = att // 128  # ATT chunks of 128
    n0 = min(128, n)
    n1 = n - n0  # second n block (68 for N=196)
    npairs = bc // 2

    nc = bacc.Bacc("TRN2", target_bir_lowering=False)

    h_d = nc.dram_tensor("h", [bc, rnn], f32, kind="ExternalInput")
    roi_d = nc.dram_tensor("roi_feats", [bc, n, rnn], f32, kind="ExternalInput")
    ctx_d = nc.dram_tensor("context_feat", [bc, rnn], f32, kind="ExternalInput")
    wvT_d = nc.dram_tensor("wvT16", [128, rc, att], f16, kind="ExternalInput")
    whT_d = nc.dram_tensor("whT16", [128, rc, att], f16, kind="ExternalInput")
    waT_d = nc.dram_tensor("waT16", [128, ac], f16, kind="ExternalInput")
    biasT_d = nc.dram_tensor("biasT", [128, ac], f32, kind="ExternalInput")
    eye16_d = nc.dram_tensor("eye16", [128, 128], f16, kind="ExternalInput")
    eye32_d = nc.dram_tensor("eye32", [128, 128], f32, kind="ExternalInput")

    comp_d = nc.dram_tensor("comp", [bc, rnn], f32, kind="ExternalOutput")
    wout_d = nc.dram_tensor("weight", [bc, n], f32, kind="ExternalOutput")

    with tile.TileContext(nc) as tc:
        with tc.tile_pool(name="consts", bufs=1) as consts:
          with (
            tc.tile_pool(name="prep", bufs=1) as prep,
            tc.tile_pool(name="ppre", bufs=2, space="PSUM") as ppre,
          ):
            eye16 = consts.tile([128, 128], f16)
            eye32 = consts.tile([128, 128], f32)
            wvT = consts.tile([128, rc, att], f16)
            whT = consts.tile([128, rc, att], f16)
            waT = consts.tile([128, ac], f16)
            biasT = consts.tile([128, ac], f32)
            ones16 = consts.tile([1, 128], f16)
            nc.vector.memset(ones16[:], 1.0)
            nc.sync.dma_start(eye16[:], eye16_d[:])
            nc.scalar.dma_start(eye32[:], eye32_d[:])
            nc.scalar.dma_start(wvT[:], wvT_d[:])
            nc.scalar.dma_start(whT[:], whT_d[:])
            nc.scalar.dma_start(waT[:], waT_d[:])
            nc.scalar.dma_start(biasT[:], biasT_d[:])

            # ---- preamble: hT16/cT16 (fp16, [128, rc, bc]) and cT32 ([128, bc, rc]) ----
            h_sb = prep.tile([bc, rnn], f32)
            c_sb = consts.tile([bc, rnn], f32)
            nc.scalar.dma_start(h_sb[:], h_d[:])
            nc.scalar.dma_start(c_sb[:], ctx_d[:])
            h16 = prep.tile([bc, rnn], f16)
            c16 = prep.tile([bc, rnn], f16)
            nc.vector.tensor_copy(h16[:], h_sb[:])
            nc.vector.tensor_copy(c16[:], c_sb[:])

            hT16 = consts.tile([128, rc, bc], f16)
            cT16 = consts.tile([128, rc, bc], f16)
            cT32 = consts.tile([128, bc, rc], f32)
            for c in range(rc):
                pt_h = ppre.tile([128, bc], f16, tag="pt_h")
                nc.tensor.transpose(pt_h[:], h16[:, 128 * c : 128 * (c + 1)], eye16[:bc, :bc])
                nc.scalar.copy(hT16[:, c, :], pt_h[:])
                pt_c = ppre.tile([128, bc], f16, tag="pt_c")
                nc.tensor.transpose(pt_c[:], c16[:, 128 * c : 128 * (c + 1)], eye16[:bc, :bc])
                nc.scalar.copy(cT16[:, c, :], pt_c[:])
                pt_c32 = ppre.tile([128, bc], f32, tag="pt_c32")
                nc.tensor.transpose(pt_c32[:], c_sb[:, 128 * c : 128 * (c + 1)], eye32[:bc, :bc])
                nc.scalar.copy(cT32[:, :, c], pt_c32[:])

            # ---- pre_T[a_chunk i][p, b] = (wh@h^T + wv@c^T)[128i+p, b] + biasT[p, i] ----
            preT = consts.tile([128, ac, bc], f32)
            for i in range(ac):
                pp = ppre.tile([128, bc], f32, tag="pp")
                for c in range(rc):
                    nc.tensor.matmul(
                        pp[:], whT[:, c, 128 * i : 128 * (i + 1)], hT16[:, c, :],
                        start=(c == 0), stop=False,
                    )
                for c in range(rc):
                    nc.tensor.matmul(
                        pp[:], wvT[:, c, 128 * i : 128 * (i + 1)], cT16[:, c, :],
                        start=False, stop=(c == rc - 1),
                    )
                nc.scalar.activation(
                    preT[:, i, :], pp[:], AFT.Identity,
                    bias=biasT[:, i : i + 1], scale=1.0,
                )

          with (
            tc.tile_pool(name="nat", bufs=4) as natp,
            tc.tile_pool(name="nat16", bufs=8) as natp16,
            tc.tile_pool(name="work", bufs=2) as work,
            tc.tile_pool(name="small", bufs=5) as small,
            tc.tile_pool(name="pmm", bufs=2, space="PSUM") as pmm,
            tc.tile_pool(name="ppt", bufs=2, space="PSUM") as ppt,
            tc.tile_pool(name="pmisc", bufs=1, space="PSUM") as pmisc,
          ):
            cast_eng = nc.gpsimd if gs_cast else nc.vector
            segs = [(0, n0, 0), (n0, n1, 1), (n, n0, 2), (n + n0, n1, 3)]

            # chunks per psum bank for the transpose staging tile
            GRP = max(1, min(rc, 2048 // (2 * n * 2)))

            def emit_A(k):
                """load + cast + transpose + stage1 + tanh + scores + softmax."""
                roiT = work.tile([128, rc, 2 * n], f16, tag="roiT", name="roiT")
                # one DMA per n-block for the whole pair: [p, bl, r]
                nat0 = natp.tile([n0, 2, rnn], f32, tag="nat0", name="nat0")
                nc.sync.dma_start(
                    nat0[:], roi_d[2 * k : 2 * k + 2, 0:n0, :].rearrange("b p r -> p b r")
                )
                nat0_16 = natp16.tile([n0, 2, rnn], f16, tag="nat0_16", name="nat0_16")
                cast_eng.tensor_copy(nat0_16[:], nat0[:])
                nat1_16 = None
                if n1 > 0:
                    nat1 = natp.tile([n1, 2, rnn], f32, tag="nat1", name="nat1")
                    nc.sync.dma_start(
                        nat1[:], roi_d[2 * k : 2 * k + 2, n0:n, :].rearrange("b p r -> p b r")
                    )
                    nat1_16 = natp16.tile([n1, 2, rnn], f16, tag="nat1_16", name="nat1_16")
                    cast_eng.tensor_copy(nat1_16[:], nat1[:])
                nat16s = (nat0_16, nat1_16)
                for bl in range(2):
                    for g in range(0, rc, GRP):
                        ng = min(GRP, rc - g)
                        pt = ppt.tile([128, GRP, n], f16, tag="pt", name="pt")
                        for cc in range(ng):
                            c = g + cc
                            nc.tensor.transpose(
                                pt[:, cc, 0:n0],
                                nat0_16[:, bl, 128 * c : 128 * (c + 1)],
                                eye16[:n0, :n0],
                            )
                            if n1 > 0:
                                nc.tensor.transpose(
                                    pt[:, cc, n0:n],
                                    nat1_16[:, bl, 128 * c : 128 * (c + 1)],
                                    eye16[:n1, :n1],
                                )
                        dst = roiT[:, g : g + ng, n * bl : n * (bl + 1)]
                        if (bl + g) % 2 == 0:
                            nc.scalar.copy(dst, pt[:, 0:ng, :])
                        else:
                            nc.vector.tensor_copy(dst, pt[:, 0:ng, :])

                # stage 1: wvroiT[i] = wv_chunk_i @ roiT (accumulate over rc)
                th = work.tile([128, ac, 2 * n], f16, tag="th", name="th")
                for i in range(ac):
                    mm = pmm.tile([128, 2 * n], f32, tag="mm", name="mm")
                    for c in range(rc):
                        nc.tensor.matmul(
                            mm[:], wvT[:, c, 128 * i : 128 * (i + 1)], roiT[:, c, :],
                            start=(c == 0), stop=(c == rc - 1),
                        )
                    for bl in range(2):
                        b = 2 * k + bl
                        nc.scalar.activation(
                            th[:, i, n * bl : n * (bl + 1)],
                            mm[:, n * bl : n * (bl + 1)],
                            AFT.Tanh, bias=preT[:, i, b : b + 1], scale=-1.0,
                        )

                # scores[1, (b, n)] = sum_a wa[a] * tanh[a, (b, n)]
                ps = pmisc.tile([1, 2 * n], f32, tag="ps", name="ps")
                for i in range(ac):
                    nc.tensor.matmul(
                        ps[:], waT[:, i : i + 1], th[:, i, :],
                        start=(i == 0), stop=(i == ac - 1),
                    )

                # softmax over n (both batches along the free dim)
                negmax = small.tile([1, 2], f32, tag="negmax", name="negmax")
                nc.vector.tensor_reduce(
                    negmax[:], ps[:].rearrange("p (b n) -> p b n", b=2),
                    X, ALU.max, negate=True,
                )
                shifted = small.tile([1, 2 * n], f32, tag="shifted", name="shifted")
                nc.vector.tensor_tensor(
                    shifted[:].rearrange("p (b n) -> p b n", b=2),
                    ps[:].rearrange("p (b n) -> p b n", b=2),
                    negmax[:].broadcast_to([1, 2, n]),
                    op=ALU.add,
                )
                ex = small.tile([1, 2 * n], f32, tag="ex", name="ex")
                nc.scalar.activation(ex[:], shifted[:], AFT.Exp)
                ssum = small.tile([1, 2], f32, tag="ssum", name="ssum")
                nc.vector.tensor_reduce(
                    ssum[:], ex[:].rearrange("p (b n) -> p b n", b=2), X, ALU.add
                )
                rinv = small.tile([1, 2], f32, tag="rinv", name="rinv")
                nc.vector.reciprocal(rinv[:], ssum[:])
                wpair = small.tile([1, 2 * n], f32, tag="wpair", name="wpair")
                nc.vector.tensor_tensor(
                    wpair[:].rearrange("p (b n) -> p b n", b=2),
                    ex[:].rearrange("p (b n) -> p b n", b=2),
                    rinv[:].broadcast_to([1, 2, n]),
                    op=ALU.mult,
                )
                nc.scalar.dma_start(
                    wout_d[2 * k : 2 * k + 2, :].rearrange("b n -> (b n)"),
                    wpair[:],
                )
                w16 = small.tile([1, 2 * n], f16, tag="w16", name="w16")
                nc.vector.tensor_copy(w16[:], wpair[:])
                return (k, nat16s, w16)

            def emit_B(st):
                """weighted sum of roi on PE (transposed out) + comp output."""
                k, nat16s, w16 = st
                # transpose the 4 weight segments onto partitions (psum cols)
                pw = pmisc.tile([128, 4, 2], f16, tag="pw", name="pw")
                for s0, ln, j in segs:
                    nc.tensor.transpose(
                        pw[0:ln, j, 0:1], w16[0:1, s0 : s0 + ln],
                        eye16[0:1, 0:1],
                    )
                wsb = small.tile([128, 4], f16, tag="wsb", name="wsb")
                for s0, ln, j in segs:
                    nc.vector.tensor_copy(wsb[0:ln, j : j + 1], pw[0:ln, j, 0:1])

                # wroiT[:, bl, c] = nat16[bl][:, c-chunk].T @ w_col[bl]
                wroiT = pmisc.tile([128, 2, rc], f32, tag="wroiT", name="wroiT")
                nat0_16, nat1_16 = nat16s
                for bl in range(2):
                    for c in range(rc):
                        nc.tensor.matmul(
                            wroiT[:, bl, c : c + 1],
                            nat0_16[:, bl, 128 * c : 128 * (c + 1)],
                            wsb[0:n0, 2 * bl : 2 * bl + 1],
                            start=True, stop=(n1 == 0),
                        )
                        if n1 > 0:
                            nc.tensor.matmul(
                                wroiT[:, bl, c : c + 1],
                                nat1_16[:, bl, 128 * c : 128 * (c + 1)],
                                wsb[0:n1, 2 * bl + 1 : 2 * bl + 2],
                                start=False, stop=True,
                            )

                # comp^T = c^T - wroi^T ; transpose back and store
                compT = small.tile([128, 2 * rc], f32, tag="compT", name="compT")
                nc.vector.tensor_tensor(
                    compT[:].rearrange("p (b c) -> p b c", b=2),
                    cT32[:, 2 * k : 2 * k + 2, :], wroiT[:], op=ALU.subtract,
                )
                pct = pmisc.tile([2 * rc, 128], f32, tag="pct", name="pct")
                nc.tensor.transpose(pct[:], compT[:], eye32[:])
                comp_sb = small.tile([2 * rc, 128], f32, tag="comp_sb", name="comp_sb")
                nc.scalar.copy(comp_sb[:], pct[:])
                nc.scalar.dma_start(
                    comp_d[2 * k : 2 * k + 2, :].rearrange("b (c d) -> (b c) d", d=128),
                    comp_sb[:],
                )

            def emit_all():
                pending = []
                for k in [kk for _ in range(repeat) for kk in range(npairs)]:
                    st = emit_A(k)
                    pending.append(st)
                    if len(pending) > 2:
                        emit_B(pending.pop(0))
                while pending:
                    emit_B(pending.pop(0))

            if hw_loop:
                with tc.For_i(0, hw_loop, 1):
                    emit_all()
            else:
                emit_all()

    nc.compile()
    return nc


def _pack_weights(wh_w, wh_b, wv_w, wv_b, wa_w):
    rc, ac = RNN // 128, ATT // 128
    # wT16[p, c, a] = w[a, 128c + p]
    wvT16 = np.ascontiguousarray(
        wv_w.T.reshape(rc, 128, ATT).transpose(1, 0, 2).astype(np.float16)
    )
    whT16 = np.ascontiguousarray(
        wh_w.T.reshape(rc, 128, ATT).transpose(1, 0, 2).astype(np.float16)
    )
    # waT16[p, i] = wa[0, 128i + p]
    waT16 = np.ascontiguousarray(wa_w[0].reshape(ac, 128).T.astype(np.float16))
    biasT = np.ascontiguousarray(
        (wh_b + wv_b).reshape(ac, 128).T.astype(np.float32)
    )
    eye16 = np.eye(128, dtype=np.float16)
    eye32 = np.eye(128, dtype=np.float32)
    return dict(
        wvT16=wvT16, whT16=whT16, waT16=waT16, biasT=biasT,
        eye16=eye16, eye32=eye32,
    )


def _get_runner():
    """Build the bass program once and return a cached jitted runner."""
    global _RUNNER
    if _RUNNER is not None:
        return _RUNNER

    import jax
    import numpy as _np
    from jax.sharding import Mesh, PartitionSpec
    from jax.experimental.shard_map import shard_map
    import concourse.mybir as mybir
    from concourse import bass2jax

    nc = _build_program()
    bass2jax.install_neuronx_cc_hook()

    partition_name = nc.partition_id_tensor.name if nc.partition_id_tensor else None

    in_names: list[str] = []
    out_names: list[str] = []
    out_avals = []
    zero_shapes = []
    for alloc in nc.m.functions[0].allocations:
        if not isinstance(alloc, mybir.MemoryLocationSet):
            continue
        name = alloc.memorylocations[0].name
        if alloc.kind == "ExternalInput":
            if name != partition_name:
                in_names.append(name)
        elif alloc.kind == "ExternalOutput":
            out_names.append(name)
            shape = tuple(alloc.tensor_shape)
            dtype = mybir.dt.np(alloc.dtype)
            out_avals.append(jax.core.ShapedArray(shape, dtype))
            zero_shapes.append((shape, dtype))

    n_params = len(in_names)
    n_outs = len(out_names)
    all_names = tuple(in_names + out_names)
    if partition_name is not None:
        all_names = all_names + (partition_name,)

    # Everything is sharded on axis 0 (run_bass_via_pjrt-style): batch inputs
    # are already global; replicated weights get tiled 8x on axis 0 in run().
    batch_inputs = {"h", "roi_feats", "context_feat"}

    def _body(*args):
        operands = list(args)
        if partition_name is not None:
            operands.append(bass2jax.partition_id_tensor())
        outs = bass2jax._bass_exec_p.bind(
            *operands,
            out_avals=tuple(out_avals),
            in_names=all_names,
            out_names=tuple(out_names),
            lowering_input_output_aliases=(),
            sim_require_finite=True,
            sim_require_nnan=True,
            nc=nc,
        )
        return tuple(outs)

    devices = jax.devices()[:NCORES]
    mesh = Mesh(_np.asarray(devices), ("core",))
    in_specs = (PartitionSpec("core"),) * (n_params + n_outs)
    out_specs = (PartitionSpec("core"),) * n_outs
    donate = tuple(range(n_params, n_params + n_outs))
    sharded = jax.jit(
        shard_map(_body, mesh=mesh, in_specs=in_specs, out_specs=out_specs,
                  check_rep=False),
        donate_argnums=donate, keep_unused=True,
    )

    def _global_args(feed: dict):
        args = []
        for nm in in_names:
            a = feed[nm]
            if nm not in batch_inputs:
                a = _np.concatenate([a] * NCORES, axis=0)
            args.append(a)
        return args

    def run(feed: dict):
        zeros = [
            _np.zeros((NCORES * s[0], *s[1:]), dt) for (s, dt) in zero_shapes
        ]
        outs = sharded(*_global_args(feed), *zeros)
        return {nm: _np.asarray(o) for nm, o in zip(out_names, outs)}

    def bench(feed: dict, iters: int = 10):
        """Time steady-state executions with device-resident inputs."""
        import time

        dev_args = [jax.device_put(a) for a in _global_args(feed)]
        def one():
            zeros = [
                _np.zeros((NCORES * s[0], *s[1:]), dt) for (s, dt) in zero_shapes
            ]
            outs = sharded(*dev_args, *zeros)
            jax.block_until_ready(outs)
            return outs

        one()  # warm
        times = []
        for _ in range(iters):
            t0 = time.perf_counter()
            one()
            times.append(time.perf_counter() - t0)
        return min(times), times

    run.bench = bench
    _RUNNER = run
    return run


def kernel(h, roi_feats, context_feat, wh_w, wh_b, wv_w, wv_b, wa_w, wa_b):
    h = np.ascontiguousarray(np.asarray(h, dtype=np.float32))
    roi_feats = np.ascontiguousarray(np.asarray(roi_feats, dtype=np.float32))
    context_feat = np.ascontiguousarray(np.asarray(context_feat, dtype=np.float32))

    feed = _pack_weights(
        np.asarray(wh_w, np.float32), np.asarray(wh_b, np.float32),
        np.asarray(wv_w, np.float32), np.asarray(wv_b, np.float32),
        np.asarray(wa_w, np.float32),
    )
    feed["h"] = h
    feed["roi_feats"] = roi_feats
    feed["context_feat"] = context_feat

    run = _get_runner()
    outs = run(feed)
    return outs["comp"], outs["weight"]


# revision 34
# speedup vs baseline: 354.8174x; 1.0716x over previous
"""CompositionAttention Trainium2 kernel.

comp_feat, weight = f(h, roi_feats, context_feat, wh, wv, wa)

Math (per batch b):
  feats[b,n,:] = context[b] - roi[b,n]
  scores[b,n]  = wa . tanh( (h[b] @ wh^T + wh_b + wv_b + context[b] @ wv^T)
                            - roi[b,n] @ wv^T )          (+ wa_b, dropped: softmax-invariant)
  weight[b]    = softmax_n(scores[b])
  comp[b]      = context[b] - sum_n weight[b,n] * roi[b,n]   (since sum_n weight = 1)

Sharding: pure data parallel, batch 256 -> 32 per core across 8 cores.
Weight matrices are packed host-side (transpose + fp16 cast) and replicated.
"""

import sys

if "/opt/trn_rl_repo" not in sys.path:
    sys.path.insert(0, "/opt/trn_rl_repo")

import numpy as np

B, N, RNN, ATT = 256, 196, 1024, 512
NCORES = 8
BC = B // NCORES  # 32 batches per core

_RUNNER = None


def _build_program(bc=BC, n=N, rnn=RNN, att=ATT, gs_cast=True, gs_bcast=True,
                   repeat=1, hw_loop=0):
    import concourse.bacc as bacc
    import concourse.mybir as mybir
    import concourse.tile as tile

    f32 = mybir.dt.float32
    f16 = mybir.dt.float16
    AFT = mybir.ActivationFunctionType
    ALU = mybir.AluOpType
    X = mybir.AxisListType.X

    rc = rnn // 128  # RNN chunks of 128
    ac = att // 128  # ATT chunks of 128
    n0 = min(128, n)
    n1 = n - n0  # second n block (68 for N=196)
    npairs = bc // 2

    nc = bacc.Bacc("TRN2", target_bir_lowering=False)

    h_d = nc.dram_tensor("h", [bc, rnn], f32, kind="ExternalInput")
    roi_d = nc.dram_tensor("roi_feats", [bc, n, rnn], f32, kind="ExternalInput")
    ctx_d = nc.dram_tensor("context_feat", [bc, rnn], f32, kind="ExternalInput")
    wvT_d = nc.dram_tensor("wvT16", [128, rc, att], f16, kind="ExternalInput")
    whT_d = nc.dram_tensor("whT16", [128, rc, att], f16, kind="ExternalInput")
    waT_d = nc.dram_tensor("waT16", [128, ac], f16, kind="ExternalInput")
    biasT_d = nc.dram_tensor("biasT", [128, ac], f32, kind="ExternalInput")
    eye16_d = nc.dram_tensor("eye16", [128, 128], f16, kind="ExternalInput")
    eye32_d = nc.dram_tensor("eye32", [128, 128], f32, kind="ExternalInput")

    comp_d = nc.dram_tensor("comp", [bc, rnn], f32, kind="ExternalOutput")
    wout_d = nc.dram_tensor("weight", [bc, n], f32, kind="ExternalOutput")

    with tile.TileContext(nc) as tc:
        with tc.tile_pool(name="consts", bufs=1) as consts:
          with (
            tc.tile_pool(name="prep", bufs=1) as prep,
            tc.tile_pool(name="ppre", bufs=2, space="PSUM") as ppre,
          ):
            eye16 = consts.tile([128, 128], f16)
            eye32 = consts.tile([128, 128], f32)
            wvT = consts.tile([128, rc, att], f16)
            whT = consts.tile([128, rc, att], f16)
            waT = consts.tile([128, ac], f16)
            biasT = consts.tile([128, ac], f32)
            ones16 = consts.tile([1, 128], f16)
            nc.vector.memset(ones16[:], 1.0)
            nc.sync.dma_start(eye16[:], eye16_d[:])
            nc.scalar.dma_start(eye32[:], eye32_d[:])
            nc.scalar.dma_start(wvT[:], wvT_d[:])
            nc.scalar.dma_start(whT[:], whT_d[:])
            nc.scalar.dma_start(waT[:], waT_d[:])
            nc.scalar.dma_start(biasT[:], biasT_d[:])

            # ---- preamble: hT16/cT16 (fp16, [128, rc, bc]) and cT32 ([128, bc, rc]) ----
            h_sb = prep.tile([bc, rnn], f32)
            c_sb = consts.tile([bc, rnn], f32)
            nc.scalar.dma_start(h_sb[:], h_d[:])
            nc.scalar.dma_start(c_sb[:], ctx_d[:])
            h16 = prep.tile([bc, rnn], f16)
            c16 = prep.tile([bc, rnn], f16)
            nc.vector.tensor_copy(h16[:], h_sb[:])
            nc.vector.tensor_copy(c16[:], c_sb[:])

            hT16 = consts.tile([128, rc, bc], f16)
            cT16 = consts.tile([128, rc, bc], f16)
            cT32 = consts.tile([128, bc, rc], f32)
            for c in range(rc):
                pt_h = ppre.tile([128, bc], f16, tag="pt_h")
                nc.tensor.transpose(pt_h[:], h16[:, 128 * c : 128 * (c + 1)], eye16[:bc, :bc])
                nc.scalar.copy(hT16[:, c, :], pt_h[:])
                pt_c = ppre.tile([128, bc], f16, tag="pt_c")
                nc.tensor.transpose(pt_c[:], c16[:, 128 * c : 128 * (c + 1)], eye16[:bc, :bc])
                nc.scalar.copy(cT16[:, c, :], pt_c[:])
                pt_c32 = ppre.tile([128, bc], f32, tag="pt_c32")
                nc.tensor.transpose(pt_c32[:], c_sb[:, 128 * c : 128 * (c + 1)], eye32[:bc, :bc])
                nc.scalar.copy(cT32[:, :, c], pt_c32[:])

            # ---- pre_T[a_chunk i][p, b] = (wh@h^T + wv@c^T)[128i+p, b] + biasT[p, i] ----
            preT = consts.tile([128, ac, bc], f32)
            for i in range(ac):
                pp = ppre.tile([128, bc], f32, tag="pp")
                for c in range(rc):
                    nc.tensor.matmul(
                        pp[:], whT[:, c, 128 * i : 128 * (i + 1)], hT16[:, c, :],
                        start=(c == 0), stop=False,
                    )
                for c in range(rc):
                    nc.tensor.matmul(
                        pp[:], wvT[:, c, 128 * i : 128 * (i + 1)], cT16[:, c, :],
                        start=False, stop=(c == rc - 1),
                    )
                nc.scalar.activation(
                    preT[:, i, :], pp[:], AFT.Identity,
                    bias=biasT[:, i : i + 1], scale=1.0,
                )

          with (
            tc.tile_pool(name="nat", bufs=4) as natp,
            tc.tile_pool(name="nat16", bufs=8) as natp16,
            tc.tile_pool(name="work", bufs=2) as work,
            tc.tile_pool(name="small", bufs=5) as small,
            tc.tile_pool(name="pmm", bufs=2, space="PSUM") as pmm,
            tc.tile_pool(name="ppt", bufs=2, space="PSUM") as ppt,
            tc.tile_pool(name="pmisc", bufs=1, space="PSUM") as pmisc,
          ):
            cast_eng = nc.gpsimd if gs_cast else nc.vector
            segs = [(0, n0, 0), (n0, n1, 1), (n, n0, 2), (n + n0, n1, 3)]

            # chunks per psum bank for the transpose staging tile
            GRP = max(1, min(rc, 2048 // (2 * n * 2)))

            def emit_A(k):
                """load + cast + transpose + stage1 + tanh + scores + softmax."""
                roiT = work.tile([128, rc, 2 * n], f16, tag="roiT", name="roiT")
                # one DMA per n-block for the whole pair: [p, bl, r]
                nat0 = natp.tile([n0, 2, rnn], f32, tag="nat0", name="nat0")
                nc.sync.dma_start(
                    nat0[:], roi_d[2 * k : 2 * k + 2, 0:n0, :].rearrange("b p r -> p b r")
                )
                nat0_16 = natp16.tile([n0, 2, rnn], f16, tag="nat0_16", name="nat0_16")
                cast_eng.tensor_copy(nat0_16[:], nat0[:])
                nat1_16 = None
                if n1 > 0:
                    nat1 = natp.tile([n1, 2, rnn], f32, tag="nat1", name="nat1")
                    nc.sync.dma_start(
                        nat1[:], roi_d[2 * k : 2 * k + 2, n0:n, :].rearrange("b p r -> p b r")
                    )
                    nat1_16 = natp16.tile([n1, 2, rnn], f16, tag="nat1_16", name="nat1_16")
                    cast_eng.tensor_copy(nat1_16[:], nat1[:])
                nat16s = (nat0_16, nat1_16)
                groups = [(bl, g) for g in range(0, rc, GRP) for bl in range(2)]
                for gi, (bl, g) in enumerate(groups):
                    ng = min(GRP, rc - g)
                    pt = ppt.tile([128, GRP, n], f16, tag="pt", name="pt")
                    for cc in range(ng):
                        c = g + cc
                        nc.tensor.transpose(
                            pt[:, cc, 0:n0],
                            nat0_16[:, bl, 128 * c : 128 * (c + 1)],
                            eye16[:n0, :n0],
                        )
                        if n1 > 0:
                            nc.tensor.transpose(
                                pt[:, cc, n0:n],
                                nat1_16[:, bl, 128 * c : 128 * (c + 1)],
                                eye16[:n1, :n1],
                            )
                    dst = roiT[:, g : g + ng, n * bl : n * (bl + 1)]
                    if gi % 2 == 0:
                        nc.scalar.copy(dst, pt[:, 0:ng, :])
                    else:
                        nc.vector.tensor_copy(dst, pt[:, 0:ng, :])

                # stage 1: wvroiT[i] = wv_chunk_i @ roiT (accumulate over rc)
                th = work.tile([128, ac, 2 * n], f16, tag="th", name="th")
                for i in range(ac):
                    mm = pmm.tile([128, 2 * n], f32, tag="mm", name="mm")
                    for c in range(rc):
                        nc.tensor.matmul(
                            mm[:], wvT[:, c, 128 * i : 128 * (i + 1)], roiT[:, c, :],
                            start=(c == 0), stop=(c == rc - 1),
                        )
                    for bl in range(2):
                        b = 2 * k + bl
                        nc.scalar.activation(
                            th[:, i, n * bl : n * (bl + 1)],
                            mm[:, n * bl : n * (bl + 1)],
                            AFT.Tanh, bias=preT[:, i, b : b + 1], scale=-1.0,
                        )

                # scores[1, (b, n)] = sum_a wa[a] * tanh[a, (b, n)]
                ps = pmisc.tile([1, 2 * n], f32, tag="ps", name="ps")
                for i in range(ac):
                    nc.tensor.matmul(
                        ps[:], waT[:, i : i + 1], th[:, i, :],
                        start=(i == 0), stop=(i == ac - 1),
                    )

                # softmax over n (both batches along the free dim)
                negmax = small.tile([1, 2], f32, tag="negmax", name="negmax")
                nc.vector.tensor_reduce(
                    negmax[:], ps[:].rearrange("p (b n) -> p b n", b=2),
                    X, ALU.max, negate=True,
                )
                shifted = small.tile([1, 2 * n], f32, tag="shifted", name="shifted")
                nc.vector.tensor_tensor(
                    shifted[:].rearrange("p (b n) -> p b n", b=2),
                    ps[:].rearrange("p (b n) -> p b n", b=2),
                    negmax[:].broadcast_to([1, 2, n]),
                    op=ALU.add,
                )
                ex = small.tile([1, 2 * n], f32, tag="ex", name="ex")
                nc.scalar.activation(ex[:], shifted[:], AFT.Exp)
                ssum = small.tile([1, 2], f32, tag="ssum", name="ssum")
                nc.vector.tensor_reduce(
                    ssum[:], ex[:].rearrange("p (b n) -> p b n", b=2), X, ALU.add
                )
                rinv = small.tile([1, 2], f32, tag="rinv", name="rinv")
                nc.vector.reciprocal(rinv[:], ssum[:])
                wpair = small.tile([1, 2 * n], f32, tag="wpair", name="wpair")
                nc.vector.tensor_tensor(
                    wpair[:].rearrange("p (b n) -> p b n", b=2),
                    ex[:].rearrange("p (b n) -> p b n", b=2),
                    rinv[:].broadcast_to([1, 2, n]),
                    op=ALU.mult,
                )
                nc.scalar.dma_start(
                    wout_d[2 * k : 2 * k + 2, :].rearrange("b n -> (b n)"),
                    wpair[:],
                )
                w16 = small.tile([1, 2 * n], f16, tag="w16", name="w16")
                nc.vector.tensor_copy(w16[:], wpair[:])
                return (k, nat16s, w16)

            def emit_B(st):
                """weighted sum of roi on PE (transposed out) + comp output."""
                k, nat16s, w16 = st
                # transpose the 4 weight segments onto partitions (psum cols)
                pw = pmisc.tile([128, 4, 2], f16, tag="pw", name="pw")
                for s0, ln, j in segs:
                    nc.tensor.transpose(
                        pw[0:ln, j, 0:1], w16[0:1, s0 : s0 + ln],
                        eye16[0:1, 0:1],
                    )
                wsb = small.tile([128, 4], f16, tag="wsb", name="wsb")
                for s0, ln, j in segs:
                    nc.vector.tensor_copy(wsb[0:ln, j : j + 1], pw[0:ln, j, 0:1])

                # wroiT[:, bl, c] = nat16[bl][:, c-chunk].T @ w_col[bl]
                wroiT = pmisc.tile([128, 2, rc], f32, tag="wroiT", name="wroiT")
                nat0_16, nat1_16 = nat16s
                for bl in range(2):
                    for c in range(rc):
                        nc.tensor.matmul(
                            wroiT[:, bl, c : c + 1],
                            nat0_16[:, bl, 128 * c : 128 * (c + 1)],
                            wsb[0:n0, 2 * bl : 2 * bl + 1],
                            start=True, stop=(n1 == 0),
                        )
                        if n1 > 0:
                            nc.tensor.matmul(
                                wroiT[:, bl, c : c + 1],
                                nat1_16[:, bl, 128 * c : 128 * (c + 1)],
                                wsb[0:n1, 2 * bl + 1 : 2 * bl + 2],
                                start=False, stop=True,
                            )

                # comp^T = c^T - wroi^T ; transpose back and store
                compT = small.tile([128, 2 * rc], f32, tag="compT", name="compT")
                nc.vector.tensor_tensor(
                    compT[:].rearrange("p (b c) -> p b c", b=2),
                    cT32[:, 2 * k : 2 * k + 2, :], wroiT[:], op=ALU.subtract,
                )
                pct = pmisc.tile([2 * rc, 128], f32, tag="pct", name="pct")
                nc.tensor.transpose(pct[:], compT[:], eye32[:])
                comp_sb = small.tile([2 * rc, 128], f32, tag="comp_sb", name="comp_sb")
                nc.scalar.copy(comp_sb[:], pct[:])
                nc.scalar.dma_start(
                    comp_d[2 * k : 2 * k + 2, :].rearrange("b (c d) -> (b c) d", d=128),
                    comp_sb[:],
                )

            def emit_all():
                pending = []
                for k in [kk for _ in range(repeat) for kk in range(npairs)]:
                    st = emit_A(k)
                    pending.append(st)
                    if len(pending) > 2:
                        emit_B(pending.pop(0))
                while pending:
                    emit_B(pending.pop(0))

            if hw_loop:
                with tc.For_i(0, hw_loop, 1):
                    emit_all()
            else:
                emit_all()

    nc.compile()
    return nc


def _pack_weights(wh_w, wh_b, wv_w, wv_b, wa_w):
    rc, ac = RNN // 128, ATT // 128
    # wT16[p, c, a] = w[a, 128c + p]
    wvT16 = np.ascontiguousarray(
        wv_w.T.reshape(rc, 128, ATT).transpose(1, 0, 2).astype(np.float16)
    )
    whT16 = np.ascontiguousarray(
        wh_w.T.reshape(rc, 128, ATT).transpose(1, 0, 2).astype(np.float16)
    )
    # waT16[p, i] = wa[0, 128i + p]
    waT16 = np.ascontiguousarray(wa_w[0].reshape(ac, 128).T.astype(np.float16))
    biasT = np.ascontiguousarray(
        (wh_b + wv_b).reshape(ac, 128).T.astype(np.float32)
    )
    eye16 = np.eye(128, dtype=np.float16)
    eye32 = np.eye(128, dtype=np.float32)
    return dict(
        wvT16=wvT16, whT16=whT16, waT16=waT16, biasT=biasT,
        eye16=eye16, eye32=eye32,
    )


def _get_runner():
    """Build the bass program once and return a cached jitted runner."""
    global _RUNNER
    if _RUNNER is not None:
        return _RUNNER

    import jax
    import numpy as _np
    from jax.sharding import Mesh, PartitionSpec
    from jax.experimental.shard_map import shard_map
    import concourse.mybir as mybir
    from concourse import bass2jax

    nc = _build_program()
    bass2jax.install_neuronx_cc_hook()

    partition_name = nc.partition_id_tensor.name if nc.partition_id_tensor else None

    in_names: list[str] = []
    out_names: list[str] = []
    out_avals = []
    zero_shapes = []
    for alloc in nc.m.functions[0].allocations:
        if not isinstance(alloc, mybir.MemoryLocationSet):
            continue
        name = alloc.memorylocations[0].name
        if alloc.kind == "ExternalInput":
            if name != partition_name:
                in_names.append(name)
        elif alloc.kind == "ExternalOutput":
            out_names.append(name)
            shape = tuple(alloc.tensor_shape)
            dtype = mybir.dt.np(alloc.dtype)
            out_avals.append(jax.core.ShapedArray(shape, dtype))
            zero_shapes.append((shape, dtype))

    n_params = len(in_names)
    n_outs = len(out_names)
    all_names = tuple(in_names + out_names)
    if partition_name is not None:
        all_names = all_names + (partition_name,)

    # Everything is sharded on axis 0 (run_bass_via_pjrt-style): batch inputs
    # are already global; replicated weights get tiled 8x on axis 0 in run().
    batch_inputs = {"h", "roi_feats", "context_feat"}

    def _body(*args):
        operands = list(args)
        if partition_name is not None:
            operands.append(bass2jax.partition_id_tensor())
        outs = bass2jax._bass_exec_p.bind(
            *operands,
            out_avals=tuple(out_avals),
            in_names=all_names,
            out_names=tuple(out_names),
            lowering_input_output_aliases=(),
            sim_require_finite=True,
            sim_require_nnan=True,
            nc=nc,
        )
        return tuple(outs)

    devices = jax.devices()[:NCORES]
    mesh = Mesh(_np.asarray(devices), ("core",))
    in_specs = (PartitionSpec("core"),) * (n_params + n_outs)
    out_specs = (PartitionSpec("core"),) * n_outs
    donate = tuple(range(n_params, n_params + n_outs))
    sharded = jax.jit(
        shard_map(_body, mesh=mesh, in_specs=in_specs, out_specs=out_specs,
                  check_rep=False),
        donate_argnums=donate, keep_unused=True,
    )

    def _global_args(feed: dict):
        args = []
        for nm in in_names:
            a = feed[nm]
            if nm not in batch_inputs:
                a = _np.concatenate([a] * NCORES, axis=0)
            args.append(a)
        return args

    def run(feed: dict):
        zeros = [
            _np.zeros((NCORES * s[0], *s[1:]), dt) for (s, dt) in zero_shapes
        ]
        outs = sharded(*_global_args(feed), *zeros)
        return {nm: _np.asarray(o) for nm, o in zip(out_names, outs)}

    def bench(feed: dict, iters: int = 10):
        """Time steady-state executions with device-resident inputs."""
        import time

        dev_args = [jax.device_put(a) for a in _global_args(feed)]
        def one():
            zeros = [
                _np.zeros((NCORES * s[0], *s[1:]), dt) for (s, dt) in zero_shapes
            ]
            outs = sharded(*dev_args, *zeros)
            jax.block_until_ready(outs)
            return outs

        one()  # warm
        times = []
        for _ in range(iters):
            t0 = time.perf_counter()
            one()
            times.append(time.perf_counter() - t0)
        return min(times), times

    run.bench = bench
    _RUNNER = run
    return run


def kernel(h, roi_feats, context_feat, wh_w, wh_b, wv_w, wv_b, wa_w, wa_b):
    h = np.ascontiguousarray(np.asarray(h, dtype=np.float32))
    roi_feats = np.ascontiguousarray(np.asarray(roi_feats, dtype=np.float32))
    context_feat = np.ascontiguousarray(np.asarray(context_feat, dtype=np.float32))

    feed = _pack_weights(
        np.asarray(wh_w, np.float32), np.asarray(wh_b, np.float32),
        np.asarray(wv_w, np.float32), np.asarray(wv_b, np.float32),
        np.asarray(wa_w, np.float32),
    )
    feed["h"] = h
    feed["roi_feats"] = roi_feats
    feed["context_feat"] = context_feat

    run = _get_runner()
    outs = run(feed)
    return outs["comp"], outs["weight"]
